# revision 31
# baseline (speedup 1.0000x reference)
"""Transformer-XL attention on 8 Trainium2 NeuronCores (Bass/Tile) — v2.

Sharding: 8 cores = 4 batches x 2 head-groups of 8 heads.
Each core computes its (batch, head-group) attention output projected through
its W_proj row-slice; host sums the two head-group partials per batch and adds
the bias terms (b_v @ W_proj + b_proj) once.

v2 structure (vs baseline):
- Position scores kept as bf16 LOGITS through the skew+transpose path; they
  are added into the content-score PSUM via an identity matmul on the PE, so
  the Activation engine does only ONE exp per score entry (halves Act load).
- Projections are chunked by tokens (double-buffered 1MB loads) and overlap
  with the first two heads' position-score (2a) pipelines.
- Queue assignment: SP = input loads + transposes, Act = exp only,
  Pool/gpsimd = skews + pad memsets + evict DMAs, DVE = psum evictions.
"""

import sys

for _p in ("/opt/trn_rl_repo",):
    if _p not in sys.path:
        sys.path.insert(0, _p)

from contextlib import ExitStack

import ml_dtypes
import numpy as np

import concourse.bacc as bacc
import concourse.bass as bass
import concourse.mybir as mybir
import concourse.tile as tile
from concourse.bass_utils import run_bass_kernel_spmd

CUR, FULL, BS, D = 1024, 2048, 4, 1024
HN, HD = 16, 64
PREV = FULL - CUR
SCALE = 1.0 / HD**0.5
HC = 8          # heads per core
CW = HC * HD    # 512 channel columns per core
BF = mybir.dt.bfloat16
F32 = mybir.dt.float32
EXP = mybir.ActivationFunctionType.Exp
BF_NP = ml_dtypes.bfloat16
GTP = 16 * 8 * 128   # GT tile row length (cols)
NEG = -50000.0       # logit pad -> exp == 0

_CACHE = {}


def _ap(t, off, dims):
    return bass.AP(tensor=t.tensor, offset=t.offset + off, ap=dims)


def _blk(d, rowlen, nblk):
    """DRAM [nblk*128, rowlen] viewed as [p, blk, col]."""
    return _ap(d, 0, [[rowlen, 128], [128 * rowlen, nblk], [1, rowlen]])


def build_program():
    nc = bacc.Bacc("TRN2", target_bir_lowering=False, debug=False)

    XcT = nc.dram_tensor("XcT", [D, CUR], BF, kind="ExternalInput").ap()
    XfT = nc.dram_tensor("XfT", [D, FULL], BF, kind="ExternalInput").ap()
    PosT = nc.dram_tensor("PosT", [D, FULL], BF, kind="ExternalInput").ap()
    Wq = nc.dram_tensor("Wq", [D, CW], BF, kind="ExternalInput").ap()
    Wk = nc.dram_tensor("Wk", [D, CW], BF, kind="ExternalInput").ap()
    Wv = nc.dram_tensor("Wv", [D, CW], BF, kind="ExternalInput").ap()
    Wpos = nc.dram_tensor("Wpos", [D, CW], BF, kind="ExternalInput").ap()
    Wproj = nc.dram_tensor("Wproj", [CW, D], BF, kind="ExternalInput").ap()
    Ident = nc.dram_tensor("Ident", [128, 128], BF, kind="ExternalInput").ap()
    b_all_d = nc.dram_tensor("b_all", [4 * CW, 1], F32, kind="ExternalInput").ap()
    out_d = nc.dram_tensor("out_part", [CUR, D], F32, kind="ExternalOutput").ap()
    z_dram = nc.dram_tensor("z_scratch", [HC, CUR], F32).ap()

    with tile.TileContext(nc) as tc, ExitStack() as ctx:
        persist = ctx.enter_context(tc.tile_pool(name="persist", bufs=1))
        ps_pool = ctx.enter_context(tc.tile_pool(name="ps", bufs=3, space="PSUM"))
        av_pool = ctx.enter_context(tc.tile_pool(name="avps", bufs=2, space="PSUM"))

        QuT = persist.tile([128, 4 * CUR], BF, tag="QuT")
        QvT = persist.tile([128, 4 * CUR], BF, tag="QvT")
        KT = persist.tile([128, 4 * FULL], BF, tag="KT")
        RT = persist.tile([128, 4 * FULL], BF, tag="RT")
        Vp = persist.tile([128, 16 * 8 * 66], BF, tag="Vp")
        OT = persist.tile([128, 4 * CUR], BF, tag="OT")
        Id_sb = persist.tile([128, 128], BF, tag="Id")
        biases = persist.tile([128, 16], F32, tag="biases")


        # ones columns of V' (col 64 of each 66-wide head slot)
        nc.vector.memset(_ap(Vp, 64, [[16 * 8 * 66, 128], [8 * 66, 16], [66, 8], [1, 1]]), 1.0)

        # 2a pools (outlive phase 1: used for lookahead heads during phase C)
        gpool = ctx.enter_context(tc.tile_pool(name="g", bufs=3))
        gspool = ctx.enter_context(tc.tile_pool(name="gs", bufs=3))
        gtpool = ctx.enter_context(tc.tile_pool(name="gt", bufs=2))

        gts = {}
        pend_tr = []

        def flush_tr(keep=0):
            """Emit pending GT transposes, keeping at most `keep` queued."""
            while len(pend_tr) > keep:
                h, qt, Gs, Wj, nblk = pend_tr.pop(0)
                nc.sync.dma_start_transpose(
                    out=_ap(gts[h], qt * 2048, [[GTP, 128], [128, nblk], [1, 128]]),
                    in_=Gs[:, 0:Wj],
                )

        def do_2a_iter(h, qt):
            """Position logits for (head h, query tile qt) -> GT[h] (bf16)."""
            ct = h // 2
            rb = (h % 2) * 64
            i0 = qt * 128
            m_lo = 896 - i0
            W = FULL - m_lo            # 1152 + i0
            Wj = i0 + 1152             # valid j width (multiple of 128)
            nblk = qt + 9
            Re = W + 128
            G = gpool.tile([128, Re], BF, tag="G")
            nc.gpsimd.memset(G[:, W:W + 128], NEG)
            off = 0
            ci = 0
            while off < W:
                wc = min(1024, W - off)
                gps = ps_pool.tile([128, 1024], F32, tag="ps")
                sc = 0
                while sc < wc:
                    wn = min(512, wc - sc)
                    nc.tensor.matmul(
                        gps[:, sc:sc + wn],
                        QvT[rb:rb + 64, ct * CUR + i0: ct * CUR + i0 + 128],
                        RT[rb:rb + 64, ct * FULL + m_lo + off + sc:
                           ct * FULL + m_lo + off + sc + wn],
                        start=True, stop=True,
                    )
                    sc += wn
                # psum eviction split ~3:1 between DVE and Act to shorten the
                # per-head 2a pipeline
                if (qt * 2 + ci) % 8 in (2, 5, 7):
                    nc.scalar.copy(G[:, off:off + wc], gps[:, 0:wc])
                else:
                    nc.vector.tensor_copy(G[:, off:off + wc], gps[:, 0:wc])
                ci += 1
                off += wc
            Gs = gspool.tile([128, 2176], BF, tag="Gs")
            # skew: Gs[p, j] = G[p, 127 + j - p]  (SP hardware DGE — cheap issue)
            nc.sync.dma_start(
                out=Gs[:, 0:Wj],
                in_=_ap(G, 127, [[Re - 1, 128], [1, Wj]]),
            )
            # transpose into GT[p, qt, t, f] = Gs[f, t*128 + p] — deferred via
            # flush_tr so it never waits at the SP queue head for its skew
            pend_tr.append((h, qt, Gs, Wj, nblk))

        # ---------------- Phase 1: projections (chunked by tokens) ----------
        with (
            tc.tile_pool(name="xp", bufs=3) as xpool,
            tc.tile_pool(name="wp1", bufs=1) as wpool,
        ):
            def load_w(dram, wtag="w"):
                w_sb = wpool.tile([128, 8 * CW], BF, tag=wtag)
                nc.sync.dma_start(
                    out=w_sb.rearrange("p (kt c) -> p kt c", kt=8),
                    in_=_blk(dram, CW, 8))
                return w_sb

            def load_xchunk(dram, c, rowlen):
                xch = xpool.tile([128, 8 * 512], BF, tag="x")
                nc.sync.dma_start(
                    out=xch.rearrange("p (kt t) -> p kt t", kt=8),
                    in_=_ap(dram, c * 512,
                            [[rowlen, 128], [128 * rowlen, 8], [1, 512]]))
                return xch

            def proj_chunk(w_sb, xch, out_sbs, bias_cols, out_off):
                """project one 512-token chunk -> out_sbs[:, ct*rowlen + out_off]"""
                for ct in range(4):
                    ps = ps_pool.tile([128, 1024], F32, tag="ps")
                    for kt in range(8):
                        nc.tensor.matmul(
                            ps[:, 0:512],
                            w_sb[:, kt * CW + ct * 128: kt * CW + ct * 128 + 128],
                            xch[:, kt * 512:(kt + 1) * 512],
                            start=(kt == 0), stop=(kt == 7),
                        )
                    for o_sb, rowlen, bcol in zip(*out_sbs, bias_cols):
                        nc.vector.tensor_scalar(
                            o_sb[:, ct * rowlen + out_off: ct * rowlen + out_off + 512],
                            ps[:, 0:512],
                            biases[:, bcol * 4 + ct: bcol * 4 + ct + 1],
                            None, mybir.AluOpType.add,
                        )

            # Q projection (2 chunks); wq + first x chunk loads go first so
            # the PE can start ASAP, then the small bias/identity loads
            wq_sb = load_w(Wq, "w")
            xc0 = load_xchunk(XcT, 0, CUR)
            nc.sync.dma_start(
                out=_ap(biases, 0, [[16, 128], [4, 4], [1, 4]]),
                in_=_ap(b_all_d, 0, [[1, 128], [CW, 4], [128, 4]]))
            xc1 = load_xchunk(XcT, 1, CUR)
            wpos_sb = load_w(Wpos, "w2")
            nc.sync.dma_start(out=Id_sb, in_=Ident)
            for c, xch in enumerate((xc0, xc1)):
                proj_chunk(wq_sb, xch, ([QuT, QvT], [CUR, CUR]), [0, 1], c * 512)
            # R projection (4 chunks)
            wk_sb = load_w(Wk, "w")
            for c in range(4):
                xch = load_xchunk(PosT, c, FULL)
                proj_chunk(wpos_sb, xch, ([RT], [FULL]), [3], c * 512)

            # prefetch first two xf chunks + Wv before the 2a(0) transposes
            # occupy the SP queue (avoids head-of-line blocking the K loads)
            wv_sb = load_w(Wv, "w2")
            xf_pre = [load_xchunk(XfT, 0, FULL), load_xchunk(XfT, 1, FULL)]

            # K + V projection (4 chunks) with 2a(0) interleaved; the later xf
            # chunk loads are issued before the 2a skews hit the DMA queue
            gt0 = gtpool.tile([128, GTP], BF, tag="GT")
            gts[0] = gt0
            QTS1 = {0: (7, 6, 5), 1: (4, 3, 2), 2: (1, 0)}
            for c in range(4):
                if c < 2:
                    xch = xf_pre[c]
                    xf_pre.append(load_xchunk(XfT, c + 2, FULL))
                else:
                    xch = xf_pre[c]
                proj_chunk(wk_sb, xch, ([KT], [FULL]), [2], c * 512)
                # V: natural layout, 4 token-tiles of 128 per chunk
                for ti in range(4):
                    tt = 4 * c + ti
                    ps = ps_pool.tile([128, 1024], F32, tag="ps")
                    for kt in range(8):
                        nc.tensor.matmul(
                            ps[:, 0:512],
                            xch[:, kt * 512 + ti * 128: kt * 512 + ti * 128 + 128],
                            wv_sb[:, kt * CW: kt * CW + CW],
                            start=(kt == 0), stop=(kt == 7),
                        )
                    nc.vector.tensor_copy(
                        _ap(Vp, tt * 8 * 66, [[16 * 8 * 66, 128], [66, 8], [1, 64]]),
                        ps[:, 0:512].rearrange("p (h d) -> p h d", h=8),
                    )
                for qt in QTS1.get(c, ()):
                    do_2a_iter(0, qt)
                    flush_tr(keep=2)
            flush_tr()

        # ---------------- Phase 2: attention ----------------
        with (
            tc.tile_pool(name="e", bufs=3) as epool,
            tc.tile_pool(name="st", bufs=2) as stpool,
            tc.tile_pool(name="zr", bufs=1) as zrpool,
            tc.tile_pool(name="ob", bufs=2) as obpool,
            tc.tile_pool(name="wp", bufs=1) as wppool,
        ):
            Wproj_sb = wppool.tile([128, 4 * D], BF, tag="Wproj")
            nc.sync.dma_start(out=Wproj_sb.rearrange("p (ct d) -> p ct d", ct=4),
                              in_=_blk(Wproj, D, 4))

            def do_2b_iter(h, t, avs, pend):
                """content + logit-add + exp + (pipelined) AV."""
                ct = h // 2
                rb = (h % 2) * 64
                qt_min = max(0, t - 8)
                ioff = qt_min * 128
                w = CUR - ioff
                cps = ps_pool.tile([128, 1024], F32, tag="ps")
                sc = 0
                while sc < w:
                    wn = min(512, w - sc)
                    nc.tensor.matmul(
                        cps[:, sc:sc + wn],
                        KT[rb:rb + 64, ct * FULL + t * 128: ct * FULL + t * 128 + 128],
                        QuT[rb:rb + 64, ct * CUR + ioff + sc: ct * CUR + ioff + sc + wn],
                        start=True, stop=False,
                    )
                    # add position logits: cps += I.T @ GT slice
                    nc.tensor.matmul(
                        cps[:, sc:sc + wn],
                        Id_sb,
                        _ap(gts[h], (qt_min + sc // 128) * 2048 + t * 128,
                            [[GTP, 128], [2048, wn // 128], [1, 128]]),
                        start=False, stop=True,
                    )
                    sc += wn
                E = epool.tile([128, 1024], BF, tag="E")
                nc.scalar.activation(E[:, 0:w], cps[:, 0:w], EXP, scale=SCALE)
                # flush previous iteration's AV, then queue this one
                if pend:
                    flush_av(*pend)
                return (h, t, E, ioff, avs)

            def flush_av(h, t, E, ioff, avs):
                # t runs 15 -> 0: chunk c=1 first accumulates at t=15, c=0 at
                # t=11 (its first valid tile); both stop at t=0
                for c in range(2):
                    lo = max(ioff, c * 512)
                    hi = (c + 1) * 512
                    if lo >= hi:
                        continue
                    first_t = 11 if c == 0 else 15
                    nc.tensor.matmul(
                        avs[c][:, lo - c * 512: hi - c * 512],
                        Vp[:, t * 8 * 66 + h * 66: t * 8 * 66 + h * 66 + 65],
                        E[:, lo - ioff: hi - ioff],
                        start=(t == first_t), stop=(t == 0),
                    )

            def do_evict(h, avs):
                """Write exp-sum row to DRAM, broadcast back, normalize the
                attention output while evicting it to OT (bf16)."""
                ct = h // 2
                rb = (h % 2) * 64
                zst = stpool.tile([128, CUR], F32, tag="zst")
                for c in range(2):
                    nc.vector.tensor_copy(
                        zst[64:65, c * 512:(c + 1) * 512], avs[c][64:65, :])
                raws = []
                for c in range(2):
                    raw = stpool.tile([64, 512], F32, tag=f"raw{c}")
                    nc.vector.tensor_copy(raw, avs[c][0:64, :])
                    raws.append(raw)
                nc.sync.dma_start(
                    out=_ap(z_dram, h * CUR, [[CUR, 1], [1, CUR]]),
                    in_=zst[64:65, :],
                )
                zrec = zrpool.tile([64, CUR], F32, tag="zrec")
                nc.sync.dma_start(
                    out=zrec, in_=_ap(z_dram, h * CUR, [[0, 64], [1, CUR]]))
                zri = zrpool.tile([64, CUR], F32, tag="zri")
                nc.vector.reciprocal(zri, zrec)
                for c in range(2):
                    ost = stpool.tile([64, 512], BF, tag="ost")
                    nc.gpsimd.tensor_mul(
                        ost, raws[c], zri[:, c * 512:(c + 1) * 512])
                    nc.gpsimd.dma_start(
                        out=OT[rb:rb + 64, ct * CUR + c * 512: ct * CUR + c * 512 + 512],
                        in_=ost,
                    )

            open_groups = []

            def close_group(gr):
                pps, it, dc = gr
                ct = 3
                nc.tensor.matmul(
                    pps[:, 0:512],
                    OT[:, ct * CUR + it * 128: ct * CUR + it * 128 + 128],
                    Wproj_sb[:, ct * D + dc * 512: ct * D + dc * 512 + 512],
                    start=False, stop=True,
                )
                osb = obpool.tile([128, 512], F32, tag="osb")
                if (it + dc) % 2 == 0:
                    nc.vector.tensor_copy(osb, pps[:, 0:512])
                else:
                    nc.scalar.copy(osb, pps[:, 0:512])
                nc.sync.dma_start(
                    out=out_d[it * 128:(it + 1) * 128, dc * 512:(dc + 1) * 512],
                    in_=osb,
                )

            for h in range(HC):
                if h + 1 < HC:
                    gt_next = gtpool.tile([128, GTP], BF, tag="GT")
                    gts[h + 1] = gt_next
                av0 = av_pool.tile([65, 512], F32, tag="av")
                av1 = av_pool.tile([65, 512], F32, tag="av")
                avs = (av0, av1)
                pend = None
                for k, t in enumerate(range(15, -1, -1)):
                    if h + 1 < HC:
                        if k < 8:
                            do_2a_iter(h + 1, 7 - k)
                        flush_tr(keep=2 if k < 7 else (1 if k == 7 else 0))
                    pend = do_2b_iter(h, t, avs, pend)
                flush_av(*pend)
                flush_tr()
                if h == HC - 1:
                    # front-run phase-3 partial groups (ct 0-2 need only heads
                    # 0-5) so the PE has work while evict(7) drains
                    for it in range(2):
                        pps = ps_pool.tile([128, 1024], F32, tag="ps")
                        for ct in range(3):
                            nc.tensor.matmul(
                                pps[:, 0:512],
                                OT[:, ct * CUR + it * 128: ct * CUR + it * 128 + 128],
                                Wproj_sb[:, ct * D + 0: ct * D + 512],
                                start=(ct == 0), stop=False,
                            )
                        open_groups.append((pps, it, 0))
                do_evict(h, avs)
                gts.pop(h)

            # ---------------- Phase 3: output projection --------------------
            done = {(it, 0) for it in range(2)}
            for it in range(8):
                for dc in range(2):
                    if (it, dc) in done:
                        continue
                    pps = ps_pool.tile([128, 1024], F32, tag="ps")
                    for ct in range(3):
                        nc.tensor.matmul(
                            pps[:, 0:512],
                            OT[:, ct * CUR + it * 128: ct * CUR + it * 128 + 128],
                            Wproj_sb[:, ct * D + dc * 512: ct * D + dc * 512 + 512],
                            start=(ct == 0), stop=False,
                        )
                    open_groups.append((pps, it, dc))
                    if len(open_groups) == 3:
                        close_group(open_groups.pop(0))
            for gr in open_groups:
                close_group(gr)

    nc.compile()
    return nc


def _prep_core_inputs(inputs, pos_embedding, full_input, u, v,
                      W_kv, b_kv, W_q, b_q, W_pos, b_pos, W_proj):
    """Host-side shard prep: returns list of 8 in_maps."""
    bf = BF_NP
    posT = np.ascontiguousarray(pos_embedding[:, 0, :].T).astype(bf)
    ident = np.eye(128, dtype=np.float32).astype(bf)
    in_maps = []
    for c in range(8):
        b, hg = c // 2, c % 2
        s = slice(hg * CW, (hg + 1) * CW)
        hs = slice(hg * HC, (hg + 1) * HC)
        in_maps.append({
            "XcT": np.ascontiguousarray(inputs[:, b, :].T).astype(bf),
            "XfT": np.ascontiguousarray(full_input[:, b, :].T).astype(bf),
            "PosT": posT,
            "Wq": np.ascontiguousarray(W_q[:, s]).astype(bf),
            "Wk": np.ascontiguousarray(W_kv[:, :HN * HD][:, s]).astype(bf),
            "Wv": np.ascontiguousarray(W_kv[:, HN * HD:][:, s]).astype(bf),
            "Wpos": np.ascontiguousarray(W_pos[:, s]).astype(bf),
            "Wproj": np.ascontiguousarray(W_proj[s, :]).astype(bf),
            "Ident": ident,
            "b_all": np.concatenate([
                (b_q[s] + u[hs].reshape(-1)).astype(np.float32),
                (b_q[s] + v[hs].reshape(-1)).astype(np.float32),
                b_kv[:HN * HD][s].astype(np.float32),
                b_pos[s].astype(np.float32),
            ]).reshape(4 * CW, 1),
        })
    return in_maps


def kernel(inputs, pos_embedding, full_input, u, v, mask,
           W_kv, b_kv, W_q, b_q, W_pos, b_pos, W_proj, b_proj,
           _want_profile=False):
    inputs = np.asarray(inputs, np.float32)
    pos_embedding = np.asarray(pos_embedding, np.float32)
    full_input = np.asarray(full_input, np.float32)

    if "nc" not in _CACHE:
        _CACHE["nc"] = build_program()
    nc = _CACHE["nc"]

    in_maps = _prep_core_inputs(
        inputs, pos_embedding, full_input,
        np.asarray(u, np.float32), np.asarray(v, np.float32),
        np.asarray(W_kv, np.float32), np.asarray(b_kv, np.float32),
        np.asarray(W_q, np.float32), np.asarray(b_q, np.float32),
        np.asarray(W_pos, np.float32), np.asarray(b_pos, np.float32),
        np.asarray(W_proj, np.float32))

    res = run_bass_kernel_spmd(nc, in_maps, list(range(8)))

    b_v = np.asarray(b_kv, np.float32)[HN * HD:]
    beta = b_v @ np.asarray(W_proj, np.float32) + np.asarray(b_proj, np.float32)
    out = np.empty((CUR, BS, D), np.float32)
    for b in range(BS):
        out[:, b, :] = (res.results[2 * b]["out_part"]
                        + res.results[2 * b + 1]["out_part"] + beta)
    if _want_profile:
        return out, res
    return out


# revision 32
# speedup vs baseline: 1.2100x; 1.2100x over previous
"""Transformer-XL attention on 8 Trainium2 NeuronCores (Bass/Tile) — v2.

Sharding: 8 cores = 4 batches x 2 head-groups of 8 heads.
Each core computes its (batch, head-group) attention output projected through
its W_proj row-slice; host sums the two head-group partials per batch and adds
the bias terms (b_v @ W_proj + b_proj) once.

v2 structure (vs baseline):
- Position scores kept as bf16 LOGITS through the skew+transpose path; they
  are added into the content-score PSUM via an identity matmul on the PE, so
  the Activation engine does only ONE exp per score entry (halves Act load).
- Projections are chunked by tokens (double-buffered 1MB loads) and overlap
  with the first two heads' position-score (2a) pipelines.
- Queue assignment: SP = input loads + transposes, Act = exp only,
  Pool/gpsimd = skews + pad memsets + evict DMAs, DVE = psum evictions.
"""

import sys

for _p in ("/opt/trn_rl_repo",):
    if _p not in sys.path:
        sys.path.insert(0, _p)

from contextlib import ExitStack

import ml_dtypes
import numpy as np

import concourse.bacc as bacc
import concourse.bass as bass
import concourse.mybir as mybir
import concourse.tile as tile
from concourse.bass_utils import run_bass_kernel_spmd

CUR, FULL, BS, D = 1024, 2048, 4, 1024
HN, HD = 16, 64
PREV = FULL - CUR
SCALE = 1.0 / HD**0.5
HC = 8          # heads per core
CW = HC * HD    # 512 channel columns per core
BF = mybir.dt.bfloat16
F32 = mybir.dt.float32
EXP = mybir.ActivationFunctionType.Exp
BF_NP = ml_dtypes.bfloat16
GTP = 16 * 8 * 128   # GT tile row length (cols)
NEG = -50000.0       # logit pad -> exp == 0

_CACHE = {}


def _ap(t, off, dims):
    return bass.AP(tensor=t.tensor, offset=t.offset + off, ap=dims)


def _blk(d, rowlen, nblk):
    """DRAM [nblk*128, rowlen] viewed as [p, blk, col]."""
    return _ap(d, 0, [[rowlen, 128], [128 * rowlen, nblk], [1, rowlen]])


def build_program():
    nc = bacc.Bacc("TRN2", target_bir_lowering=False, debug=False)

    XcT = nc.dram_tensor("XcT", [D, CUR], BF, kind="ExternalInput").ap()
    XfT = nc.dram_tensor("XfT", [D, FULL], BF, kind="ExternalInput").ap()
    PosT = nc.dram_tensor("PosT", [D, FULL], BF, kind="ExternalInput").ap()
    Wq = nc.dram_tensor("Wq", [D, CW], BF, kind="ExternalInput").ap()
    Wk = nc.dram_tensor("Wk", [D, CW], BF, kind="ExternalInput").ap()
    Wv = nc.dram_tensor("Wv", [D, CW], BF, kind="ExternalInput").ap()
    Wpos = nc.dram_tensor("Wpos", [D, CW], BF, kind="ExternalInput").ap()
    Wproj = nc.dram_tensor("Wproj", [CW, D], BF, kind="ExternalInput").ap()
    Ident = nc.dram_tensor("Ident", [128, 128], BF, kind="ExternalInput").ap()
    b_all_d = nc.dram_tensor("b_all", [4 * CW, 1], F32, kind="ExternalInput").ap()
    out_d = nc.dram_tensor("out_part", [CUR, D], F32, kind="ExternalOutput").ap()
    z_dram = nc.dram_tensor("z_scratch", [HC, CUR], F32).ap()

    with tile.TileContext(nc) as tc, ExitStack() as ctx:
        persist = ctx.enter_context(tc.tile_pool(name="persist", bufs=1))
        ps_pool = ctx.enter_context(tc.tile_pool(name="ps", bufs=3, space="PSUM"))
        av_pool = ctx.enter_context(tc.tile_pool(name="avps", bufs=2, space="PSUM"))

        QuT = persist.tile([128, 4 * CUR], BF, tag="QuT")
        QvT = persist.tile([128, 4 * CUR], BF, tag="QvT")
        KT = persist.tile([128, 4 * FULL], BF, tag="KT")
        RT = persist.tile([128, 4 * FULL], BF, tag="RT")
        Vp = persist.tile([128, 16 * 8 * 66], BF, tag="Vp")
        OT = persist.tile([128, 4 * CUR], BF, tag="OT")
        Id_sb = persist.tile([128, 128], BF, tag="Id")
        biases = persist.tile([128, 16], F32, tag="biases")


        # ones columns of V' (col 64 of each 66-wide head slot)
        nc.vector.memset(_ap(Vp, 64, [[16 * 8 * 66, 128], [8 * 66, 16], [66, 8], [1, 1]]), 1.0)

        # 2a pools (outlive phase 1: used for lookahead heads during phase C)
        gpool = ctx.enter_context(tc.tile_pool(name="g", bufs=3))
        gspool = ctx.enter_context(tc.tile_pool(name="gs", bufs=3))
        gtpool = ctx.enter_context(tc.tile_pool(name="gt", bufs=2))

        gts = {}
        pend_tr = []

        def flush_tr(keep=0):
            """Emit pending GT transposes, keeping at most `keep` queued."""
            while len(pend_tr) > keep:
                h, qt, Gs, Wj, nblk = pend_tr.pop(0)
                nc.sync.dma_start_transpose(
                    out=_ap(gts[h], qt * 2048, [[GTP, 128], [128, nblk], [1, 128]]),
                    in_=Gs[:, 0:Wj],
                )

        def do_2a_iter(h, qt):
            """Position logits for (head h, query tile qt) -> GT[h] (bf16)."""
            ct = h // 2
            rb = (h % 2) * 64
            i0 = qt * 128
            m_lo = 896 - i0
            W = FULL - m_lo            # 1152 + i0
            Wj = i0 + 1152             # valid j width (multiple of 128)
            nblk = qt + 9
            Re = W + 128
            G = gpool.tile([128, Re], BF, tag="G")
            nc.gpsimd.memset(G[:, W:W + 128], NEG)
            off = 0
            ci = 0
            while off < W:
                wc = min(1024, W - off)
                gps = ps_pool.tile([128, 1024], F32, tag="ps")
                sc = 0
                while sc < wc:
                    wn = min(512, wc - sc)
                    nc.tensor.matmul(
                        gps[:, sc:sc + wn],
                        QvT[rb:rb + 64, ct * CUR + i0: ct * CUR + i0 + 128],
                        RT[rb:rb + 64, ct * FULL + m_lo + off + sc:
                           ct * FULL + m_lo + off + sc + wn],
                        start=True, stop=True,
                    )
                    sc += wn
                # psum eviction split ~3:1 between DVE and Act to shorten the
                # per-head 2a pipeline
                if (qt * 2 + ci) % 8 in (2, 5, 7):
                    nc.scalar.copy(G[:, off:off + wc], gps[:, 0:wc])
                else:
                    nc.vector.tensor_copy(G[:, off:off + wc], gps[:, 0:wc])
                ci += 1
                off += wc
            Gs = gspool.tile([128, 2176], BF, tag="Gs")
            # skew: Gs[p, j] = G[p, 127 + j - p]  (SP hardware DGE — cheap issue)
            nc.sync.dma_start(
                out=Gs[:, 0:Wj],
                in_=_ap(G, 127, [[Re - 1, 128], [1, Wj]]),
            )
            # transpose into GT[p, qt, t, f] = Gs[f, t*128 + p] — deferred via
            # flush_tr so it never waits at the SP queue head for its skew
            pend_tr.append((h, qt, Gs, Wj, nblk))

        # ---------------- Phase 1: projections (chunked by tokens) ----------
        with (
            tc.tile_pool(name="xp", bufs=3) as xpool,
            tc.tile_pool(name="wp1", bufs=1) as wpool,
        ):
            def load_w(dram, wtag="w"):
                w_sb = wpool.tile([128, 8 * CW], BF, tag=wtag)
                nc.sync.dma_start(
                    out=w_sb.rearrange("p (kt c) -> p kt c", kt=8),
                    in_=_blk(dram, CW, 8))
                return w_sb

            def load_xchunk(dram, c, rowlen):
                xch = xpool.tile([128, 8 * 512], BF, tag="x")
                nc.sync.dma_start(
                    out=xch.rearrange("p (kt t) -> p kt t", kt=8),
                    in_=_ap(dram, c * 512,
                            [[rowlen, 128], [128 * rowlen, 8], [1, 512]]))
                return xch

            def proj_chunk(w_sb, xch, out_sbs, bias_cols, out_off):
                """project one 512-token chunk -> out_sbs[:, ct*rowlen + out_off]"""
                for ct in range(4):
                    ps = ps_pool.tile([128, 1024], F32, tag="ps")
                    for kt in range(8):
                        nc.tensor.matmul(
                            ps[:, 0:512],
                            w_sb[:, kt * CW + ct * 128: kt * CW + ct * 128 + 128],
                            xch[:, kt * 512:(kt + 1) * 512],
                            start=(kt == 0), stop=(kt == 7),
                        )
                    for o_sb, rowlen, bcol in zip(*out_sbs, bias_cols):
                        nc.vector.tensor_scalar(
                            o_sb[:, ct * rowlen + out_off: ct * rowlen + out_off + 512],
                            ps[:, 0:512],
                            biases[:, bcol * 4 + ct: bcol * 4 + ct + 1],
                            None, mybir.AluOpType.add,
                        )

            # Q projection (2 chunks); wq + first x chunk loads go first so
            # the PE can start ASAP, then the small bias/identity loads
            wq_sb = load_w(Wq, "w")
            xc0 = load_xchunk(XcT, 0, CUR)
            nc.sync.dma_start(
                out=_ap(biases, 0, [[16, 128], [4, 4], [1, 4]]),
                in_=_ap(b_all_d, 0, [[1, 128], [CW, 4], [128, 4]]))
            xc1 = load_xchunk(XcT, 1, CUR)
            wpos_sb = load_w(Wpos, "w2")
            nc.sync.dma_start(out=Id_sb, in_=Ident)
            for c, xch in enumerate((xc0, xc1)):
                proj_chunk(wq_sb, xch, ([QuT, QvT], [CUR, CUR]), [0, 1], c * 512)
            # R projection (4 chunks)
            wk_sb = load_w(Wk, "w")
            for c in range(4):
                xch = load_xchunk(PosT, c, FULL)
                proj_chunk(wpos_sb, xch, ([RT], [FULL]), [3], c * 512)

            # prefetch first two xf chunks + Wv before the 2a(0) transposes
            # occupy the SP queue (avoids head-of-line blocking the K loads)
            wv_sb = load_w(Wv, "w2")
            xf_pre = [load_xchunk(XfT, 0, FULL), load_xchunk(XfT, 1, FULL)]

            # K + V projection (4 chunks) with 2a(0) interleaved; the later xf
            # chunk loads are issued before the 2a skews hit the DMA queue
            gt0 = gtpool.tile([128, GTP], BF, tag="GT")
            gts[0] = gt0
            QTS1 = {0: (0, 1, 2), 1: (3, 4, 5), 2: (6, 7)}
            for c in range(4):
                if c < 2:
                    xch = xf_pre[c]
                    xf_pre.append(load_xchunk(XfT, c + 2, FULL))
                else:
                    xch = xf_pre[c]
                proj_chunk(wk_sb, xch, ([KT], [FULL]), [2], c * 512)
                # V: natural layout, 4 token-tiles of 128 per chunk
                for ti in range(4):
                    tt = 4 * c + ti
                    ps = ps_pool.tile([128, 1024], F32, tag="ps")
                    for kt in range(8):
                        nc.tensor.matmul(
                            ps[:, 0:512],
                            xch[:, kt * 512 + ti * 128: kt * 512 + ti * 128 + 128],
                            wv_sb[:, kt * CW: kt * CW + CW],
                            start=(kt == 0), stop=(kt == 7),
                        )
                    nc.vector.tensor_copy(
                        _ap(Vp, tt * 8 * 66, [[16 * 8 * 66, 128], [66, 8], [1, 64]]),
                        ps[:, 0:512].rearrange("p (h d) -> p h d", h=8),
                    )
                for qt in QTS1.get(c, ()):
                    do_2a_iter(0, qt)
                    flush_tr(keep=2)
            flush_tr()

        # ---------------- Phase 2: attention ----------------
        with (
            tc.tile_pool(name="e", bufs=3) as epool,
            tc.tile_pool(name="st", bufs=2) as stpool,
            tc.tile_pool(name="zr", bufs=1) as zrpool,
            tc.tile_pool(name="ob", bufs=2) as obpool,
            tc.tile_pool(name="wp", bufs=1) as wppool,
        ):
            Wproj_sb = wppool.tile([128, 4 * D], BF, tag="Wproj")
            nc.sync.dma_start(out=Wproj_sb.rearrange("p (ct d) -> p ct d", ct=4),
                              in_=_blk(Wproj, D, 4))

            def do_2b_iter(h, t, avs, pend):
                """content + logit-add + exp + (pipelined) AV."""
                ct = h // 2
                rb = (h % 2) * 64
                qt_min = max(0, t - 8)
                ioff = qt_min * 128
                w = CUR - ioff
                cps = ps_pool.tile([128, 1024], F32, tag="ps")
                sc = 0
                while sc < w:
                    wn = min(512, w - sc)
                    nc.tensor.matmul(
                        cps[:, sc:sc + wn],
                        KT[rb:rb + 64, ct * FULL + t * 128: ct * FULL + t * 128 + 128],
                        QuT[rb:rb + 64, ct * CUR + ioff + sc: ct * CUR + ioff + sc + wn],
                        start=True, stop=False,
                    )
                    # add position logits: cps += I.T @ GT slice
                    nc.tensor.matmul(
                        cps[:, sc:sc + wn],
                        Id_sb,
                        _ap(gts[h], (qt_min + sc // 128) * 2048 + t * 128,
                            [[GTP, 128], [2048, wn // 128], [1, 128]]),
                        start=False, stop=True,
                    )
                    sc += wn
                E = epool.tile([128, 1024], BF, tag="E")
                nc.scalar.activation(E[:, 0:w], cps[:, 0:w], EXP, scale=SCALE)
                # flush previous iteration's AV, then queue this one
                if pend:
                    flush_av(*pend)
                return (h, t, E, ioff, avs)

            def flush_av(h, t, E, ioff, avs):
                # t runs 15 -> 0: chunk c=1 first accumulates at t=15, c=0 at
                # t=11 (its first valid tile); both stop at t=0
                for c in range(2):
                    lo = max(ioff, c * 512)
                    hi = (c + 1) * 512
                    if lo >= hi:
                        continue
                    first_t = 11 if c == 0 else 15
                    nc.tensor.matmul(
                        avs[c][:, lo - c * 512: hi - c * 512],
                        Vp[:, t * 8 * 66 + h * 66: t * 8 * 66 + h * 66 + 65],
                        E[:, lo - ioff: hi - ioff],
                        start=(t == first_t), stop=(t == 8),
                    )

            def do_evict(h, avs):
                """Write exp-sum row to DRAM, broadcast back, normalize the
                attention output while evicting it to OT (bf16)."""
                ct = h // 2
                rb = (h % 2) * 64
                zst = stpool.tile([128, CUR], F32, tag="zst")
                for c in range(2):
                    nc.vector.tensor_copy(
                        zst[64:65, c * 512:(c + 1) * 512], avs[c][64:65, :])
                raws = []
                for c in range(2):
                    raw = stpool.tile([64, 512], F32, tag=f"raw{c}")
                    nc.vector.tensor_copy(raw, avs[c][0:64, :])
                    raws.append(raw)
                nc.sync.dma_start(
                    out=_ap(z_dram, h * CUR, [[CUR, 1], [1, CUR]]),
                    in_=zst[64:65, :],
                )
                zrec = zrpool.tile([64, CUR], F32, tag="zrec")
                nc.sync.dma_start(
                    out=zrec, in_=_ap(z_dram, h * CUR, [[0, 64], [1, CUR]]))
                zri = zrpool.tile([64, CUR], F32, tag="zri")
                nc.vector.reciprocal(zri, zrec)
                for c in range(2):
                    ost = stpool.tile([64, 512], BF, tag="ost")
                    nc.gpsimd.tensor_mul(
                        ost, raws[c], zri[:, c * 512:(c + 1) * 512])
                    nc.gpsimd.dma_start(
                        out=OT[rb:rb + 64, ct * CUR + c * 512: ct * CUR + c * 512 + 512],
                        in_=ost,
                    )

            open_groups = []

            def close_group(gr):
                pps, it, dc = gr
                ct = 3
                nc.tensor.matmul(
                    pps[:, 0:512],
                    OT[:, ct * CUR + it * 128: ct * CUR + it * 128 + 128],
                    Wproj_sb[:, ct * D + dc * 512: ct * D + dc * 512 + 512],
                    start=False, stop=True,
                )
                osb = obpool.tile([128, 512], F32, tag="osb")
                if (it + dc) % 2 == 0:
                    nc.vector.tensor_copy(osb, pps[:, 0:512])
                else:
                    nc.scalar.copy(osb, pps[:, 0:512])
                nc.sync.dma_start(
                    out=out_d[it * 128:(it + 1) * 128, dc * 512:(dc + 1) * 512],
                    in_=osb,
                )

            for h in range(HC):
                if h + 1 < HC:
                    gt_next = gtpool.tile([128, GTP], BF, tag="GT")
                    gts[h + 1] = gt_next
                av0 = av_pool.tile([65, 512], F32, tag="av")
                av1 = av_pool.tile([65, 512], F32, tag="av")
                avs = (av0, av1)
                pend = None
                TORD = list(range(15, 8, -1)) + list(range(0, 9))
                QTS = {0: (0, 1), 1: (2, 3), 2: (4,), 3: (5,), 4: (6,), 5: (7,)}
                for k, t in enumerate(TORD):
                    if h + 1 < HC:
                        for qt in QTS.get(k, ()):
                            do_2a_iter(h + 1, qt)
                        flush_tr(keep=2 if k < 6 else (1 if k == 6 else 0))
                    pend = do_2b_iter(h, t, avs, pend)
                flush_av(*pend)
                flush_tr()
                if h == HC - 1:
                    # front-run phase-3 partial groups (ct 0-2 need only heads
                    # 0-5) so the PE has work while evict(7) drains
                    for it in range(2):
                        pps = ps_pool.tile([128, 1024], F32, tag="ps")
                        for ct in range(3):
                            nc.tensor.matmul(
                                pps[:, 0:512],
                                OT[:, ct * CUR + it * 128: ct * CUR + it * 128 + 128],
                                Wproj_sb[:, ct * D + 0: ct * D + 512],
                                start=(ct == 0), stop=False,
                            )
                        open_groups.append((pps, it, 0))
                do_evict(h, avs)
                gts.pop(h)

            # ---------------- Phase 3: output projection --------------------
            done = {(it, 0) for it in range(2)}
            for it in range(8):
                for dc in range(2):
                    if (it, dc) in done:
                        continue
                    pps = ps_pool.tile([128, 1024], F32, tag="ps")
                    for ct in range(3):
                        nc.tensor.matmul(
                            pps[:, 0:512],
                            OT[:, ct * CUR + it * 128: ct * CUR + it * 128 + 128],
                            Wproj_sb[:, ct * D + dc * 512: ct * D + dc * 512 + 512],
                            start=(ct == 0), stop=False,
                        )
                    open_groups.append((pps, it, dc))
                    if len(open_groups) == 3:
                        close_group(open_groups.pop(0))
            for gr in open_groups:
                close_group(gr)

    nc.compile()
    return nc


def _prep_core_inputs(inputs, pos_embedding, full_input, u, v,
                      W_kv, b_kv, W_q, b_q, W_pos, b_pos, W_proj):
    """Host-side shard prep: returns list of 8 in_maps."""
    bf = BF_NP
    posT = np.ascontiguousarray(pos_embedding[:, 0, :].T).astype(bf)
    ident = np.eye(128, dtype=np.float32).astype(bf)
    in_maps = []
    for c in range(8):
        b, hg = c // 2, c % 2
        s = slice(hg * CW, (hg + 1) * CW)
        hs = slice(hg * HC, (hg + 1) * HC)
        in_maps.append({
            "XcT": np.ascontiguousarray(inputs[:, b, :].T).astype(bf),
            "XfT": np.ascontiguousarray(full_input[:, b, :].T).astype(bf),
            "PosT": posT,
            "Wq": np.ascontiguousarray(W_q[:, s]).astype(bf),
            "Wk": np.ascontiguousarray(W_kv[:, :HN * HD][:, s]).astype(bf),
            "Wv": np.ascontiguousarray(W_kv[:, HN * HD:][:, s]).astype(bf),
            "Wpos": np.ascontiguousarray(W_pos[:, s]).astype(bf),
            "Wproj": np.ascontiguousarray(W_proj[s, :]).astype(bf),
            "Ident": ident,
            "b_all": np.concatenate([
                (b_q[s] + u[hs].reshape(-1)).astype(np.float32),
                (b_q[s] + v[hs].reshape(-1)).astype(np.float32),
                b_kv[:HN * HD][s].astype(np.float32),
                b_pos[s].astype(np.float32),
            ]).reshape(4 * CW, 1),
        })
    return in_maps


def kernel(inputs, pos_embedding, full_input, u, v, mask,
           W_kv, b_kv, W_q, b_q, W_pos, b_pos, W_proj, b_proj,
           _want_profile=False):
    inputs = np.asarray(inputs, np.float32)
    pos_embedding = np.asarray(pos_embedding, np.float32)
    full_input = np.asarray(full_input, np.float32)

    if "nc" not in _CACHE:
        _CACHE["nc"] = build_program()
    nc = _CACHE["nc"]

    in_maps = _prep_core_inputs(
        inputs, pos_embedding, full_input,
        np.asarray(u, np.float32), np.asarray(v, np.float32),
        np.asarray(W_kv, np.float32), np.asarray(b_kv, np.float32),
        np.asarray(W_q, np.float32), np.asarray(b_q, np.float32),
        np.asarray(W_pos, np.float32), np.asarray(b_pos, np.float32),
        np.asarray(W_proj, np.float32))

    res = run_bass_kernel_spmd(nc, in_maps, list(range(8)))

    b_v = np.asarray(b_kv, np.float32)[HN * HD:]
    beta = b_v @ np.asarray(W_proj, np.float32) + np.asarray(b_proj, np.float32)
    out = np.empty((CUR, BS, D), np.float32)
    for b in range(BS):
        out[:, b, :] = (res.results[2 * b]["out_part"]
                        + res.results[2 * b + 1]["out_part"] + beta)
    if _want_profile:
        return out, res
    return out
